# revision 1
# baseline (speedup 1.0000x reference)
"""Bass/Trainium2 kernel for nn_GatherUpdate: LayerNorm + Linear + per-atom
row gather + residual add, data-parallel over batch across 8 NeuronCores.

reference:
    normed = LayerNorm(s) * gamma + beta            # s: [B, 2048, 384]
    upd    = normed @ W.T                           # W: [128, 384] -> [B, 2048, 128]
    out    = atom_embed + upd[:, cond_to_s_idx, :]  # atom_embed: [B, 32768, 128]

Per-core plan (core b handles batch b):
  1. LN over 16 tiles of [128 res, 384] via bn_stats/bn_aggr.
  2. PE-transpose normed tiles (cs onto partitions), matmul against
     gamma-folded W^T chunks, accumulate in PSUM; beta folded in via a K=1
     ones-row matmul against (beta @ W.T).
  3. Write upd table [2048, 128] to DRAM scratch.
  4. dma_gather 512B rows from the table (32 calls x 1024 idxs — the HW
     SWDGE ring holds ~127 descriptors/engine — rotated over 4 SWDGE
     queues), add to atom_embed tiles, store. Atom tiles use contiguous
     4MB DMAs; the host pre-permutes the int16 index stream so the gather
     output layout matches the atom tiles.
"""

import sys

sys.path.insert(0, "/opt/trn_rl_repo")

import numpy as np

B = 8
N_ATOMS = 32768
N_RES = 2048
C_S = 384
C_ATOM = 128
EPS = 1e-5
P = 128
N_RES_TILES = N_RES // P  # 16
N_CHUNKS = 4  # atom chunks per core
CHUNK = N_ATOMS // N_CHUNKS  # 8192 atoms
CT = CHUNK // P  # 64 tiles of 128 atoms per chunk
KC = C_S // P  # 3 contraction chunks
GSUB = 1024  # idxs per dma_gather (HW SWDGE ring holds ~127 desc/engine)
GQ = CHUNK // GSUB  # 8 sub-gathers per atom chunk
GT = GSUB // P  # 8 x 128-atom tiles per sub-gather
NQUEUES = 4  # rotate SWDGE queues so desc-gen overlaps in-flight gathers

_compiled = None


def _build(repeat=1):
    """Build the per-core program. repeat>1 unrolls the whole pipeline N
    times (used only for timing: wall(N)-wall(1) cancels dispatch/transfer
    overhead)."""
    import concourse.bacc as bacc
    import concourse.tile as tile
    from concourse import mybir
    from concourse.masks import make_identity
    from concourse.tile import add_dep_helper

    f32 = mybir.dt.float32

    nc = bacc.Bacc(
        "TRN2", target_bir_lowering=False, debug=False, num_swdge_queues=NQUEUES
    )

    atom = nc.dram_tensor("atom", [N_ATOMS, C_ATOM], f32, kind="ExternalInput")
    s_in = nc.dram_tensor("s_in", [N_RES, C_S], f32, kind="ExternalInput")
    idx16 = nc.dram_tensor(
        "idx16", [P, N_ATOMS // 16], mybir.dt.int16, kind="ExternalInput"
    )
    wg = nc.dram_tensor("wg", [P, C_S], f32, kind="ExternalInput")
    wt = nc.dram_tensor("wt", [P, C_S], f32, kind="ExternalInput")
    beta3 = nc.dram_tensor("beta3", [P, KC], f32, kind="ExternalInput")
    out = nc.dram_tensor("out", [N_ATOMS, C_ATOM], f32, kind="ExternalOutput")
    upd_dram = nc.dram_tensor("upd_scratch", [N_RES, C_ATOM], f32, kind="Internal")

    with tile.TileContext(nc) as tc:
        with (
            tc.tile_pool(name="singles", bufs=1) as singles,
            tc.tile_pool(name="stats", bufs=4) as stats,
            tc.tile_pool(name="t2p", bufs=3) as t2p,
            tc.tile_pool(name="t2tp", bufs=6) as t2tp,
            tc.tile_pool(name="psum_tt", bufs=4, space="PSUM") as psum_tt,
            tc.tile_pool(name="psum_mm", bufs=2, space="PSUM") as psum_mm,
            tc.tile_pool(name="atoms", bufs=2) as atoms_pool,
            tc.tile_pool(name="gath", bufs=2) as gath_pool,
        ):
            # --- s load first: it gates LN start, so its first quarter must
            # win the DMA engines before the constants below. p-major:
            # s_big[p, j, :] = row p*16 + j -> contiguous per partition
            # (LN is per-row, so any row->lane mapping works).
            s_pt = s_in.ap().rearrange("(p t) c -> p t c", p=P)

            def load_s():
                sb = singles.tile([P, N_RES_TILES, C_S], f32, tag="s_big")
                for sq in range(4):
                    tq = N_RES_TILES // 4
                    nc.sync.dma_start(
                        out=sb[:, sq * tq : (sq + 1) * tq, :],
                        in_=s_pt[:, sq * tq : (sq + 1) * tq, :],
                    )
                return sb

            s_first = load_s()

            # --- constants / small inputs (idx last: gathers need it ~40us in)
            wg_sb = singles.tile([P, C_S], f32)
            nc.sync.dma_start(out=wg_sb[:], in_=wg.ap())
            wt_sb = singles.tile([P, C_S], f32)
            nc.sync.dma_start(out=wt_sb[:], in_=wt.ap())
            beta_sb = singles.tile([P, KC], f32)
            nc.sync.dma_start(out=beta_sb[:], in_=beta3.ap())
            idx_sb = singles.tile([P, N_ATOMS // 16], mybir.dt.int16)
            nc.sync.dma_start(out=idx_sb[:], in_=idx16.ap())

            ident = singles.tile([P, P], f32)
            make_identity(nc, ident[:])
            ones1 = singles.tile([1, P], f32)
            nc.vector.memset(ones1[:], 1.0)
            eps_t = singles.tile([P, 1], f32)
            nc.vector.memset(eps_t[:], EPS)

            # --- bias row: (beta @ W.T) [1, C_ATOM] ---
            bias_ps = psum_mm.tile([1, C_ATOM], f32, tag="biasps")
            for k in range(KC):
                nc.tensor.matmul(
                    bias_ps[:],
                    lhsT=beta_sb[:, k : k + 1],
                    rhs=wt_sb[:, k * P : (k + 1) * P],
                    start=(k == 0),
                    stop=(k == KC - 1),
                )
            bias_sb = singles.tile([1, C_ATOM], f32)
            nc.vector.tensor_copy(out=bias_sb[:], in_=bias_ps[:])
            # broadcast bias across partitions once: ones[1,P].T @ bias[1,P]
            bias_bc_ps = psum_mm.tile([P, C_ATOM], f32, tag="updps")
            nc.tensor.matmul(
                bias_bc_ps[:], lhsT=ones1[:], rhs=bias_sb[:], start=True, stop=True
            )
            bias_bc = singles.tile([P, C_ATOM], f32)
            nc.vector.tensor_copy(out=bias_bc[:], in_=bias_bc_ps[:])

            prev_gathers = []
            for _rep in range(repeat):
                s_big = s_first if _rep == 0 else load_s()

                # --- LN + matmul into upd table ---
                upd_big = singles.tile([P, N_RES_TILES, C_ATOM], f32, tag="upd_big")
                for i in range(N_RES_TILES):
                    st6 = stats.tile([P, 6], f32)
                    nc.vector.bn_stats(out=st6[:], in_=s_big[:, i, :])
                    mv = stats.tile([P, 2], f32)
                    nc.vector.bn_aggr(out=mv[:], in_=st6[:])
                    std = stats.tile([P, 1], f32)
                    nc.scalar.activation(
                        out=std[:],
                        in_=mv[:, 1:2],
                        func=mybir.ActivationFunctionType.Sqrt,
                        bias=eps_t[:],
                    )
                    rstd = stats.tile([P, 1], f32)
                    nc.vector.reciprocal(out=rstd[:], in_=std[:])
                    t2 = t2p.tile([P, C_S], f32)
                    nc.vector.tensor_scalar(
                        out=t2[:],
                        in0=s_big[:, i, :],
                        scalar1=mv[:, 0:1],
                        scalar2=rstd[:],
                        op0=mybir.AluOpType.subtract,
                        op1=mybir.AluOpType.mult,
                    )
                    t2t_chunks = []
                    for k in range(KC):
                        tt_ps = psum_tt.tile([P, P], f32)
                        nc.tensor.transpose(
                            out=tt_ps[:],
                            in_=t2[:, k * P : (k + 1) * P],
                            identity=ident[:],
                        )
                        t2t = t2tp.tile([P, P], f32)
                        nc.vector.tensor_copy(out=t2t[:], in_=tt_ps[:])
                        t2t_chunks.append(t2t)
                    upd_ps = psum_mm.tile([P, C_ATOM], f32, tag="updps")
                    for k in range(KC):
                        nc.tensor.matmul(
                            upd_ps[:],
                            lhsT=t2t_chunks[k][:],
                            rhs=wg_sb[:, k * P : (k + 1) * P],
                            start=(k == 0),
                            stop=(k == KC - 1),
                        )
                    # fold the beta bias in during the PSUM->SBUF move (DVE)
                    # instead of a 4th K=1 matmul: keeps PE on the critical
                    # path free for transposes/matmuls
                    nc.vector.tensor_tensor(
                        out=upd_big[:, i, :],
                        in0=upd_ps[:],
                        in1=bias_bc[:],
                        op=mybir.AluOpType.add,
                    )

                # store table in quarters so stores overlap later LN tiles;
                # gathers gate on all four. p-major: upd_big[p, j, :] is
                # table row p*16 + j -> contiguous per partition.
                upd_stores = []
                upd_pt = upd_dram.ap().rearrange("(p t) c -> p t c", p=P)
                for sq in range(4):
                    tq = N_RES_TILES // 4
                    us = nc.sync.dma_start(
                        out=upd_pt[:, sq * tq : (sq + 1) * tq, :],
                        in_=upd_big[:, sq * tq : (sq + 1) * tq, :],
                    )
                    upd_stores.append(us)
                # WAR: don't overwrite the table while last rep still gathers
                for pg in prev_gathers:
                    for us in upd_stores:
                        add_dep_helper(
                            us.ins, pg.ins, reason="WAR on upd table across reps"
                        )
                prev_gathers = []

                # --- gather + residual add over 4 chunks of 8192 atoms ---
                for c in range(N_CHUNKS):
                    at = atoms_pool.tile([P, CT, C_ATOM], f32, tag="at")
                    nc.sync.dma_start(
                        out=at[:],
                        in_=atom.ap()[c * CHUNK : (c + 1) * CHUNK, :].rearrange(
                            "(p t) c -> p t c", p=P
                        ),
                    )
                    g = gath_pool.tile([P, CT, C_ATOM], f32, tag="g")
                    for q in range(GQ):
                        gi = nc.gpsimd.dma_gather(
                            g[:, q * GT : (q + 1) * GT, :],
                            upd_dram.ap(),
                            idx_sb[
                                :,
                                c * (CHUNK // 16) + q * (GSUB // 16) : c * (CHUNK // 16)
                                + (q + 1) * (GSUB // 16),
                            ],
                            GSUB,
                            GSUB,
                            C_ATOM,
                            queue_num=(c * GQ + q) % NQUEUES,
                        )
                        for us in upd_stores:
                            add_dep_helper(
                                gi.ins, us.ins, reason="upd table must be in DRAM"
                            )
                        prev_gathers.append(gi)
                    nc.vector.tensor_add(out=at[:], in0=at[:], in1=g[:])
                    nc.sync.dma_start(
                        out=out.ap()[c * CHUNK : (c + 1) * CHUNK, :].rearrange(
                            "(p t) c -> p t c", p=P
                        ),
                        in_=at[:],
                    )

    nc.compile()
    return nc


def _prep_core_inputs(atom_embed, s, cond_to_s_idx, ln_gamma, ln_beta, W):
    """Host-side sharding + layout marshalling (no math beyond folding the
    LN scale into the weight layout)."""
    # gamma-folded W^T, chunked so cs-chunk k sits at free columns [k*128, ...)
    wg_full = (W * ln_gamma[None, :]).T.astype(np.float32)  # [C_S, C_ATOM]
    wg_host = np.ascontiguousarray(
        wg_full.reshape(KC, P, C_ATOM).transpose(1, 0, 2).reshape(P, C_S)
    )
    wt_full = np.ascontiguousarray(W.T.astype(np.float32))  # [C_S, C_ATOM]
    wt_host = np.ascontiguousarray(
        wt_full.reshape(KC, P, C_ATOM).transpose(1, 0, 2).reshape(P, C_S)
    )
    beta_host = np.ascontiguousarray(
        ln_beta.astype(np.float32).reshape(KC, P).T
    )  # [P, KC]

    in_maps = []
    for b in range(B):
        idxb = cond_to_s_idx[b].astype(np.int16)  # values < 2048
        # atom tile layout within a 4096-chunk: at[p, t] = atom p*CT + t.
        # sub-gather q writes g[j%128, q*GT + j//128] for list position j, so
        # position j of sub-gather q must hold the index of atom
        # (j%128)*CT + q*GT + j//128.
        A = idxb.reshape(N_CHUNKS, P, GQ, GT)  # [c, p, q, u]
        L = A.transpose(0, 2, 3, 1).reshape(N_CHUNKS, GQ, GSUB)  # j = u*128+p
        # wrap within each sub-gather: position j -> [j%16, j//16]
        Wr = L.reshape(N_CHUNKS, GQ, GSUB // 16, 16).transpose(0, 1, 3, 2)
        idx_full = np.ascontiguousarray(
            Wr.transpose(2, 0, 1, 3).reshape(16, N_ATOMS // 16)
        )
        idx_rep = np.ascontiguousarray(np.tile(idx_full, (P // 16, 1)))
        in_maps.append(
            {
                "atom": np.ascontiguousarray(atom_embed[b]),
                "s_in": np.ascontiguousarray(s[b]),
                "idx16": idx_rep,
                "wg": wg_host,
                "wt": wt_host,
                "beta3": beta_host,
            }
        )
    return in_maps


def kernel(atom_embed, s, cond_to_s_idx, ln_gamma, ln_beta, W):
    global _compiled
    from concourse.bass_utils import run_bass_kernel_spmd

    atom_embed = np.asarray(atom_embed, dtype=np.float32)
    s = np.asarray(s, dtype=np.float32)
    cond_to_s_idx = np.asarray(cond_to_s_idx)
    ln_gamma = np.asarray(ln_gamma, dtype=np.float32)
    ln_beta = np.asarray(ln_beta, dtype=np.float32)
    W = np.asarray(W, dtype=np.float32)

    if _compiled is None:
        _compiled = _build()
    in_maps = _prep_core_inputs(atom_embed, s, cond_to_s_idx, ln_gamma, ln_beta, W)
    res = run_bass_kernel_spmd(_compiled, in_maps, core_ids=list(range(B)))
    out = np.stack([res.results[b]["out"] for b in range(B)], axis=0)
    return out



# revision 8
# speedup vs baseline: 154.6006x; 154.6006x over previous
"""Bass/Trainium2 kernel for nn_GatherUpdate: LayerNorm + Linear + per-atom
row gather + residual add, data-parallel over batch across 8 NeuronCores.

reference:
    normed = LayerNorm(s) * gamma + beta            # s: [B, 2048, 384]
    upd    = normed @ W.T                           # W: [128, 384] -> [B, 2048, 128]
    out    = atom_embed + upd[:, cond_to_s_idx, :]  # atom_embed: [B, 32768, 128]

Per-core plan (core b handles batch b). Everything runs in TRANSPOSED space
(c_atom on partitions) so the gather is a within-partition free-dim gather
that runs on-chip via the GPSIMD ap_gather ucode instead of a 32768-descriptor
SWDGE row-gather from DRAM (the old approach; ~1.2us/row = 39ms):

  1. Load s^T [384, 2048] (host pre-transposed). Fold LN algebraically into
     the matmul: upd^T = rstd*(Wg^T s^T) - rstd*mu*wgsum + bias, where
     Wg = gamma-folded W^T, wgsum/bias are host-folded parameter vectors.
     Stats (mu, E[x^2]) come from ones-column matmuls on PE; the rank-1
     -wgsum*mu correction is accumulated into the same PSUM by a k=1 matmul.
  2. rstd broadcast to 128 partitions via a k=1 ones matmul; one DVE
     tensor_tensor turns PSUM into the finished upd^T table [128, 2048] in
     SBUF (1 MB, stays on-chip).
  3. ap_gather (GPSIMD) pulls upd^T[:, idx] for 4096-atom chunks straight
     from SBUF; idx stream is host-wrapped into the 16-partition format.
  4. One DVE scalar_tensor_tensor per chunk does atom^T + gathered + bias
     (the beta@W.T bias rides along as the per-partition scalar); atom I/O
     is bf16 and host-transposed, so the DMA streams are contiguous.
"""

import sys

sys.path.insert(0, "/opt/trn_rl_repo")

import numpy as np
import ml_dtypes

B = 8
N_ATOMS = 32768
N_RES = 2048
C_S = 384
C_ATOM = 128
EPS = 1e-5
P = 128
KC = C_S // P  # 3 contraction chunks
QN = 4  # 512-column chunks for stats/matmul/fixup
QS = N_RES // QN  # 512
GN = 8  # gather/add chunks
GCH = N_ATOMS // GN  # 4096 atoms per chunk
BF16 = ml_dtypes.bfloat16

_compiled = None


def _build(repeat=1):
    """Build the per-core program. repeat>1 unrolls the whole pipeline N
    times (used only for timing: wall(N)-wall(1) cancels dispatch/transfer
    overhead)."""
    import concourse.bacc as bacc
    import concourse.tile as tile
    from concourse import mybir

    f32 = mybir.dt.float32
    f32r = mybir.dt.float32r
    bf16 = mybir.dt.bfloat16
    i16 = mybir.dt.int16
    AF = mybir.ActivationFunctionType
    OP = mybir.AluOpType

    nc = bacc.Bacc("TRN2", target_bir_lowering=False, debug=False)

    sT_d = nc.dram_tensor("sT", [P, KC * N_RES], bf16, kind="ExternalInput")
    atomT_d = nc.dram_tensor("atomT", [P, N_ATOMS], bf16, kind="ExternalInput")
    idx_d = nc.dram_tensor("idx16", [P, N_ATOMS // 16], i16, kind="ExternalInput")
    wg_d = nc.dram_tensor("wg", [P, KC * C_ATOM], bf16, kind="ExternalInput")
    negw_d = nc.dram_tensor("negw", [1, C_ATOM], bf16, kind="ExternalInput")
    biasc_d = nc.dram_tensor("biasc", [P, 1], f32, kind="ExternalInput")
    outT_d = nc.dram_tensor("outT", [P, N_ATOMS], bf16, kind="ExternalOutput")

    with tile.TileContext(nc) as tc:
        with (
            tc.tile_pool(name="consts", bufs=1) as consts,
            tc.tile_pool(name="sbig", bufs=1) as sbig,
            tc.tile_pool(name="rows", bufs=1) as rows,
            tc.tile_pool(name="t1p", bufs=2) as t1p,
            tc.tile_pool(name="updp", bufs=1) as updp,
            tc.tile_pool(name="ps_stat", bufs=2, space="PSUM") as ps_stat,
            tc.tile_pool(name="ps_a", bufs=2, space="PSUM") as ps_a,
            tc.tile_pool(name="ps_b", bufs=2, space="PSUM") as ps_b,
            tc.tile_pool(name="gpool", bufs=2) as gpool,
            tc.tile_pool(name="apool", bufs=3) as apool,
            tc.tile_pool(name="opool", bufs=2) as opool,
        ):
            # --- constants (loaded once) ---
            wg_sb = consts.tile([P, KC * C_ATOM], bf16)
            nc.sync.dma_start(out=wg_sb[:], in_=wg_d.ap())
            negw_sb = consts.tile([1, C_ATOM], bf16)
            nc.sync.dma_start(out=negw_sb[:], in_=negw_d.ap())
            biasc_sb = consts.tile([P, 1], f32)
            nc.sync.dma_start(out=biasc_sb[:], in_=biasc_d.ap())
            idx_sb = consts.tile([P, N_ATOMS // 16], i16)
            nc.sync.dma_start(out=idx_sb[:], in_=idx_d.ap())
            ones1 = consts.tile([1, P], bf16)
            nc.vector.memset(ones1[:], 1.0)
            onesc = consts.tile([P, 1], bf16)
            nc.vector.memset(onesc[:], 1.0)
            eps_t = consts.tile([1, 1], f32)
            nc.vector.memset(eps_t[:], EPS)

            for _rep in range(repeat):
                # --- loads: s first (it gates the critical path) ---
                sT = sbig.tile([P, KC * N_RES], bf16, tag="sT")
                for k in range(KC):
                    nc.sync.dma_start(
                        out=sT[:, k * N_RES : (k + 1) * N_RES],
                        in_=sT_d.ap()[:, k * N_RES : (k + 1) * N_RES],
                    )
                ats = []
                for c in range(GN):
                    at = apool.tile([P, GCH], bf16, tag="at")
                    nc.sync.dma_start(
                        out=at[:], in_=atomT_d.ap()[:, c * GCH : (c + 1) * GCH]
                    )
                    ats.append(at)

                # --- squares for E[x^2] ---
                sq = sbig.tile([P, KC * N_RES], bf16, tag="sq")
                for k in range(KC):
                    nc.scalar.activation(
                        out=sq[:, k * N_RES : (k + 1) * N_RES],
                        in_=sT[:, k * N_RES : (k + 1) * N_RES],
                        func=AF.Square,
                    )

                # --- per-512-chunk pipeline: stats -> scalar chain -> A -> fixup
                mu_sb = rows.tile([1, N_RES], bf16, tag="mu")
                ex2e = rows.tile([1, N_RES], f32, tag="ex2e")
                musq = rows.tile([1, N_RES], f32, tag="musq")
                vare = rows.tile([1, N_RES], f32, tag="vare")
                stdv = rows.tile([1, N_RES], f32, tag="stdv")
                rstd = rows.tile([1, N_RES], bf16, tag="rstd")
                updT = updp.tile([P, N_RES], f32, tag="updT")

                for q in range(QN):
                    qs = slice(q * QS, (q + 1) * QS)
                    # sum(s) and sum(s^2) over c_s via ones-column matmuls
                    mu_ps = ps_stat.tile([1, QS], f32, tag="mu")
                    sq_ps = ps_stat.tile([1, QS], f32, tag="sq")
                    for k in range(KC):
                        ks = slice(k * N_RES + q * QS, k * N_RES + (q + 1) * QS)
                        nc.tensor.matmul(
                            mu_ps[:],
                            lhsT=onesc[:],
                            rhs=sT[:, ks],
                            start=(k == 0),
                            stop=(k == KC - 1),
                        )
                    for k in range(KC):
                        ks = slice(k * N_RES + q * QS, k * N_RES + (q + 1) * QS)
                        nc.tensor.matmul(
                            sq_ps[:],
                            lhsT=onesc[:],
                            rhs=sq[:, ks],
                            start=(k == 0),
                            stop=(k == KC - 1),
                        )
                    # mu = sum/384 (DVE);  musq = (sum/384)^2 (ACT, from PSUM)
                    nc.vector.tensor_scalar(
                        out=mu_sb[:, qs],
                        in0=mu_ps[:],
                        scalar1=1.0 / C_S,
                        scalar2=None,
                        op0=OP.mult,
                    )
                    nc.scalar.activation(
                        out=musq[:, qs], in_=mu_ps[:], func=AF.Square, scale=1.0 / C_S
                    )
                    # E[x^2] + eps
                    nc.scalar.activation(
                        out=ex2e[:, qs],
                        in_=sq_ps[:],
                        func=AF.Identity,
                        bias=eps_t[:],
                        scale=1.0 / C_S,
                    )
                    # var+eps = E[x^2]+eps - mu^2 ;  rstd = 1/sqrt(var+eps)
                    nc.vector.scalar_tensor_tensor(
                        out=vare[:, qs],
                        in0=musq[:, qs],
                        scalar=-1.0,
                        in1=ex2e[:, qs],
                        op0=OP.mult,
                        op1=OP.add,
                    )
                    nc.scalar.activation(out=stdv[:, qs], in_=vare[:, qs], func=AF.Sqrt)
                    with nc.allow_low_precision(reason="rstd~O(1), bf16 ok at 2e-2 tol"):
                        nc.vector.reciprocal(out=rstd[:, qs], in_=stdv[:, qs])

                    # A = Wg^T s^T - wgsum (x) mu   (rank-1 via k=1 matmul)
                    a_ps = ps_a.tile([P, QS], f32, tag="A")
                    for k in range(KC):
                        ks = slice(k * N_RES + q * QS, k * N_RES + (q + 1) * QS)
                        nc.tensor.matmul(
                            a_ps[:],
                            lhsT=wg_sb[:, k * C_ATOM : (k + 1) * C_ATOM],
                            rhs=sT[:, ks],
                            start=(k == 0),
                            stop=False,
                        )
                    nc.tensor.matmul(
                        a_ps[:],
                        lhsT=negw_sb[:],
                        rhs=mu_sb[:, qs],
                        start=False,
                        stop=True,
                    )
                    # rstd broadcast to 128 partitions (k=1 ones matmul)
                    b_ps = ps_b.tile([P, QS], f32, tag="t1b")
                    nc.tensor.matmul(
                        b_ps[:],
                        lhsT=ones1[:],
                        rhs=rstd[:, qs],
                        start=True,
                        stop=True,
                    )
                    t1b_sb = t1p.tile([P, QS], f32, tag="t1bsb")
                    nc.scalar.copy(out=t1b_sb[:], in_=b_ps[:])
                    # updT = A * rstd_bcast
                    nc.vector.tensor_tensor(
                        out=updT[:, qs], in0=a_ps[:], in1=t1b_sb[:], op=OP.mult
                    )

                # --- gather + residual add + store, 8 chunks of 4096 atoms ---
                for c in range(GN):
                    g = gpool.tile([P, GCH], f32, tag="g")
                    nc.gpsimd.ap_gather(
                        g[:],
                        updT[:],
                        idx_sb[:, c * (GCH // 16) : (c + 1) * (GCH // 16)],
                        P,
                        N_RES,
                        1,
                        GCH,
                    )
                    ot = opool.tile([P, GCH], bf16, tag="ot")
                    nc.vector.scalar_tensor_tensor(
                        out=ot[:],
                        in0=ats[c][:],
                        scalar=biasc_sb[:, 0:1],
                        in1=g[:],
                        op0=OP.add,
                        op1=OP.add,
                    )
                    nc.sync.dma_start(
                        out=outT_d.ap()[:, c * GCH : (c + 1) * GCH], in_=ot[:]
                    )

    nc.compile()
    return nc


def _prep_core_inputs(atom_embed, s, cond_to_s_idx, ln_gamma, ln_beta, W):
    """Host-side sharding + layout marshalling (transposes + folding the LN
    affine params into the weight layout)."""
    wg_full = (W * ln_gamma[None, :]).T.astype(np.float32)  # [C_S, C_ATOM]
    wg_host = np.ascontiguousarray(
        wg_full.reshape(KC, P, C_ATOM).transpose(1, 0, 2).reshape(P, KC * C_ATOM)
    ).astype(BF16)
    negw_host = np.ascontiguousarray(-wg_full.sum(axis=0).reshape(1, C_ATOM)).astype(
        BF16
    )
    biasc_host = np.ascontiguousarray(
        (W.astype(np.float32) @ ln_beta.astype(np.float32)).reshape(P, 1)
    )

    in_maps = []
    for b in range(B):
        sT = np.ascontiguousarray(s[b].T.astype(np.float32))  # [C_S, N_RES]
        sT_host = np.ascontiguousarray(
            sT.reshape(KC, P, N_RES).transpose(1, 0, 2).reshape(P, KC * N_RES)
        ).astype(BF16)
        atomT_host = np.ascontiguousarray(atom_embed[b].T).astype(BF16)
        idxb = np.asarray(cond_to_s_idx[b]).astype(np.int16)  # values < 2048
        # ap_gather 16-partition wrap: list position j comes from
        # idx[j%16, j//16] within each 16-partition group; all 8 groups
        # share the same list.
        wrapped = np.ascontiguousarray(idxb.reshape(N_ATOMS // 16, 16).T)
        idx_host = np.ascontiguousarray(np.tile(wrapped, (P // 16, 1)))
        in_maps.append(
            {
                "sT": sT_host,
                "atomT": atomT_host,
                "idx16": idx_host,
                "wg": wg_host,
                "negw": negw_host,
                "biasc": biasc_host,
            }
        )
    return in_maps


def _gather_output(res):
    out = np.empty((B, N_ATOMS, C_ATOM), dtype=np.float32)
    for b in range(B):
        out[b] = res.results[b]["outT"].astype(np.float32).T
    return out


def kernel(atom_embed, s, cond_to_s_idx, ln_gamma, ln_beta, W):
    global _compiled
    from concourse.bass_utils import run_bass_kernel_spmd

    atom_embed = np.asarray(atom_embed, dtype=np.float32)
    s = np.asarray(s, dtype=np.float32)
    cond_to_s_idx = np.asarray(cond_to_s_idx)
    ln_gamma = np.asarray(ln_gamma, dtype=np.float32)
    ln_beta = np.asarray(ln_beta, dtype=np.float32)
    W = np.asarray(W, dtype=np.float32)

    if _compiled is None:
        _compiled = _build()
    in_maps = _prep_core_inputs(atom_embed, s, cond_to_s_idx, ln_gamma, ln_beta, W)
    res = run_bass_kernel_spmd(_compiled, in_maps, core_ids=list(range(B)))
    return _gather_output(res)


# revision 11
# speedup vs baseline: 942.6954x; 6.0976x over previous
"""Bass/Trainium2 kernel for nn_GatherUpdate: LayerNorm + Linear + per-atom
row gather + residual add, data-parallel over batch across 8 NeuronCores.

reference:
    normed = LayerNorm(s) * gamma + beta            # s: [B, 2048, 384]
    upd    = normed @ W.T                           # W: [128, 384] -> [B, 2048, 128]
    out    = atom_embed + upd[:, cond_to_s_idx, :]  # atom_embed: [B, 32768, 128]

Per-core plan (core b handles batch b). Everything runs in TRANSPOSED space
(c_atom on partitions). The LN+Linear is folded algebraically into matmuls:
upd^T = rstd*(Wg^T s^T) - rstd*mu*wgsum + bias (Wg/wgsum/bias host-folded).

The gather is restructured as a sorted one-hot EXPANSION on the PE. The host
sorts atoms by residue index and lays them out in 16 windows of 2560 output
columns, window w holding atoms whose residue is in [128w, 128w+128). For a
512-column chunk the gather is then one k=128 matmul:
    g[ca, j] = sum_r upd_nat[128w+r, ca] * onehot[r, j]
with onehot[r, j] = (shifted_idx[j] == r) built by one DVE is_equal per chunk
from a host-replicated fp16 shifted-index stream. The atom_embed residual is
accumulated into the same PSUM by an identity matmul, and the beta@W.T bias
rides the ACT PSUM->SBUF drain as a per-partition bias. Window overflow
(>2560 atoms on one 128-residue window; never happens for uniform indices)
falls back to a 512-slot GPSIMD ap_gather cleanup chunk.

This replaces the v1 on-chip ap_gather of all 32768 atoms (27.3 ns/idx ucode
= ~900 us/core measured) with ~35 us of PE work; measured HW total for v1 was
966 us vs 39 ms for the SWDGE row-gather baseline.
"""

import sys

sys.path.insert(0, "/opt/trn_rl_repo")

import numpy as np
import ml_dtypes

B = 8
N_ATOMS = 32768
N_RES = 2048
C_S = 384
C_ATOM = 128
EPS = 1e-5
P = 128
KC = C_S // P  # 3 contraction chunks
QN = 4  # 512-column chunks for stats/matmul/fixup
QS = N_RES // QN  # 512
NW = 16  # residue windows
WR = N_RES // NW  # 128 residues per window
LW = 2560  # output columns per window (2048 mean + 11.6 sigma slack)
LC = LW // 512  # 5 chunks of 512 per window
CL = 512  # cleanup (overflow) columns, handled by ap_gather
NCOLS = NW * LW + CL  # 41472
BF16 = ml_dtypes.bfloat16

_compiled = None
_last_aux = None


def _build(repeat=1):
    """Build the per-core program. repeat>1 unrolls the whole pipeline N
    times (used only for timing: wall(N)-wall(1) cancels dispatch/transfer
    overhead)."""
    import concourse.bacc as bacc
    import concourse.tile as tile
    from concourse import mybir
    from concourse.masks import make_identity

    f32 = mybir.dt.float32
    bf16 = mybir.dt.bfloat16
    fp16 = mybir.dt.float16
    i16 = mybir.dt.int16
    AF = mybir.ActivationFunctionType
    OP = mybir.AluOpType

    nc = bacc.Bacc("TRN2", target_bir_lowering=False, debug=False)

    sT_d = nc.dram_tensor("sT", [P, KC * N_RES], bf16, kind="ExternalInput")
    atomS_d = nc.dram_tensor("atomS", [P, NCOLS], bf16, kind="ExternalInput")
    shift_d = nc.dram_tensor("shift", [P, NW * LW], fp16, kind="ExternalInput")
    idxcl_d = nc.dram_tensor("idxcl", [P, CL // 16], i16, kind="ExternalInput")
    wg_d = nc.dram_tensor("wg", [P, KC * C_ATOM], bf16, kind="ExternalInput")
    negw_d = nc.dram_tensor("negw", [1, C_ATOM], bf16, kind="ExternalInput")
    biasc_d = nc.dram_tensor("biasc", [P, 1], f32, kind="ExternalInput")
    identb_d = nc.dram_tensor("identb", [P, P], bf16, kind="ExternalInput")
    iotac_d = nc.dram_tensor("iotac", [P, 1], f32, kind="ExternalInput")
    outS_d = nc.dram_tensor("outS", [P, NCOLS], bf16, kind="ExternalOutput")

    with tile.TileContext(nc) as tc:
        with (
            tc.tile_pool(name="consts", bufs=1) as consts,
            tc.tile_pool(name="sbig", bufs=1) as sbig,
            tc.tile_pool(name="rows", bufs=1) as rows,
            tc.tile_pool(name="t1p", bufs=2) as t1p,
            tc.tile_pool(name="updp", bufs=1) as updp,
            tc.tile_pool(name="ps_stat", bufs=1, space="PSUM") as ps_stat,
            tc.tile_pool(name="ps_a", bufs=2, space="PSUM") as ps_a,
            tc.tile_pool(name="ps_b", bufs=1, space="PSUM") as ps_b,
            tc.tile_pool(name="ps_t", bufs=1, space="PSUM") as ps_t,
            tc.tile_pool(name="ps_e", bufs=2, space="PSUM") as ps_e,
            tc.tile_pool(name="ohp", bufs=4) as ohp,
            tc.tile_pool(name="shp", bufs=3) as shp,
            tc.tile_pool(name="atp", bufs=3) as atp,
            tc.tile_pool(name="otp", bufs=3) as otp,
            tc.tile_pool(name="clp", bufs=1) as clp,
        ):
            # --- constants (loaded once) ---
            wg_sb = consts.tile([P, KC * C_ATOM], bf16)
            nc.sync.dma_start(out=wg_sb[:], in_=wg_d.ap())
            negw_sb = consts.tile([1, C_ATOM], bf16)
            nc.sync.dma_start(out=negw_sb[:], in_=negw_d.ap())
            biasc_sb = consts.tile([P, 1], f32)
            nc.sync.dma_start(out=biasc_sb[:], in_=biasc_d.ap())
            identb = consts.tile([P, P], bf16)
            nc.sync.dma_start(out=identb[:], in_=identb_d.ap())
            iotac = consts.tile([P, 1], f32)
            nc.sync.dma_start(out=iotac[:], in_=iotac_d.ap())
            idxcl_sb = consts.tile([P, CL // 16], i16)
            nc.sync.dma_start(out=idxcl_sb[:], in_=idxcl_d.ap())
            ones1 = consts.tile([1, P], bf16)
            nc.vector.memset(ones1[:], 1.0)
            onesc = consts.tile([P, 1], bf16)
            nc.vector.memset(onesc[:], 1.0)
            eps_t = consts.tile([1, 1], f32)
            nc.vector.memset(eps_t[:], EPS)
            ident32 = consts.tile([P, P], f32)
            make_identity(nc, ident32[:])

            for _rep in range(repeat):
                # --- loads: s first (it gates the critical path) ---
                sT = sbig.tile([P, KC * N_RES], bf16, tag="sT")
                for k in range(KC):
                    nc.sync.dma_start(
                        out=sT[:, k * N_RES : (k + 1) * N_RES],
                        in_=sT_d.ap()[:, k * N_RES : (k + 1) * N_RES],
                    )
                at_cl = clp.tile([P, CL], bf16, tag="atcl")
                nc.sync.dma_start(out=at_cl[:], in_=atomS_d.ap()[:, NW * LW :])

                # --- squares for E[x^2] ---
                sq = sbig.tile([P, KC * N_RES], bf16, tag="sq")
                for k in range(KC):
                    nc.scalar.activation(
                        out=sq[:, k * N_RES : (k + 1) * N_RES],
                        in_=sT[:, k * N_RES : (k + 1) * N_RES],
                        func=AF.Square,
                    )

                # --- LN folded into matmuls: updT = rstd*(Wg^T sT - wgsum x mu)
                mu_sb = rows.tile([1, N_RES], bf16, tag="mu")
                ex2e = rows.tile([1, N_RES], f32, tag="ex2e")
                musq = rows.tile([1, N_RES], f32, tag="musq")
                vare = rows.tile([1, N_RES], f32, tag="vare")
                stdv = rows.tile([1, N_RES], f32, tag="stdv")
                rstd = rows.tile([1, N_RES], bf16, tag="rstd")
                updT = updp.tile([P, N_RES], f32, tag="updT")

                for q in range(QN):
                    qs = slice(q * QS, (q + 1) * QS)
                    mu_ps = ps_stat.tile([1, QS], f32, tag="mu")
                    sq_ps = ps_stat.tile([1, QS], f32, tag="sq")
                    for k in range(KC):
                        ks = slice(k * N_RES + q * QS, k * N_RES + (q + 1) * QS)
                        nc.tensor.matmul(
                            mu_ps[:],
                            lhsT=onesc[:],
                            rhs=sT[:, ks],
                            start=(k == 0),
                            stop=(k == KC - 1),
                        )
                    for k in range(KC):
                        ks = slice(k * N_RES + q * QS, k * N_RES + (q + 1) * QS)
                        nc.tensor.matmul(
                            sq_ps[:],
                            lhsT=onesc[:],
                            rhs=sq[:, ks],
                            start=(k == 0),
                            stop=(k == KC - 1),
                        )
                    nc.vector.tensor_scalar(
                        out=mu_sb[:, qs],
                        in0=mu_ps[:],
                        scalar1=1.0 / C_S,
                        scalar2=None,
                        op0=OP.mult,
                    )
                    nc.scalar.activation(
                        out=musq[:, qs], in_=mu_ps[:], func=AF.Square, scale=1.0 / C_S
                    )
                    nc.scalar.activation(
                        out=ex2e[:, qs],
                        in_=sq_ps[:],
                        func=AF.Identity,
                        bias=eps_t[:],
                        scale=1.0 / C_S,
                    )
                    nc.vector.scalar_tensor_tensor(
                        out=vare[:, qs],
                        in0=musq[:, qs],
                        scalar=-1.0,
                        in1=ex2e[:, qs],
                        op0=OP.mult,
                        op1=OP.add,
                    )
                    nc.scalar.activation(out=stdv[:, qs], in_=vare[:, qs], func=AF.Sqrt)
                    with nc.allow_low_precision(reason="rstd~O(1), bf16 ok at 2e-2"):
                        nc.vector.reciprocal(out=rstd[:, qs], in_=stdv[:, qs])

                    a_ps = ps_a.tile([P, QS], f32, tag="A")
                    for k in range(KC):
                        ks = slice(k * N_RES + q * QS, k * N_RES + (q + 1) * QS)
                        nc.tensor.matmul(
                            a_ps[:],
                            lhsT=wg_sb[:, k * C_ATOM : (k + 1) * C_ATOM],
                            rhs=sT[:, ks],
                            start=(k == 0),
                            stop=False,
                        )
                    nc.tensor.matmul(
                        a_ps[:],
                        lhsT=negw_sb[:],
                        rhs=mu_sb[:, qs],
                        start=False,
                        stop=True,
                    )
                    b_ps = ps_b.tile([P, QS], f32, tag="t1b")
                    nc.tensor.matmul(
                        b_ps[:], lhsT=ones1[:], rhs=rstd[:, qs], start=True, stop=True
                    )
                    t1b_sb = t1p.tile([P, QS], f32, tag="t1bsb")
                    nc.scalar.copy(out=t1b_sb[:], in_=b_ps[:])
                    nc.vector.tensor_tensor(
                        out=updT[:, qs], in0=a_ps[:], in1=t1b_sb[:], op=OP.mult
                    )

                # --- overflow cleanup: ap_gather of <=512 leftover atoms ---
                g_cl = clp.tile([P, CL], f32, tag="gcl")
                nc.gpsimd.ap_gather(
                    g_cl[:], updT[:], idxcl_sb[:], P, N_RES, 1, CL
                )
                o_cl = clp.tile([P, CL], bf16, tag="ocl")
                nc.vector.scalar_tensor_tensor(
                    out=o_cl[:],
                    in0=at_cl[:],
                    scalar=biasc_sb[:, 0:1],
                    in1=g_cl[:],
                    op0=OP.add,
                    op1=OP.add,
                )
                nc.sync.dma_start(out=outS_d.ap()[:, NW * LW :], in_=o_cl[:])

                # --- transpose updT into natural-layout bf16 window weights ---
                un_all = updp.tile([P, NW * P], bf16, tag="un")
                for w in range(NW):
                    tr_ps = ps_t.tile([P, P], f32, tag="tr")
                    nc.tensor.transpose(
                        out=tr_ps[:],
                        in_=updT[:, w * WR : (w + 1) * WR],
                        identity=ident32[:],
                    )
                    nc.scalar.copy(out=un_all[:, w * P : (w + 1) * P], in_=tr_ps[:])

                # --- sorted one-hot expansion + residual add, per window ---
                for w in range(NW):
                    sh = shp.tile([P, LW], fp16, tag="sh")
                    nc.sync.dma_start(
                        out=sh[:], in_=shift_d.ap()[:, w * LW : (w + 1) * LW]
                    )
                    at = atp.tile([P, LW], bf16, tag="at")
                    nc.sync.dma_start(
                        out=at[:], in_=atomS_d.ap()[:, w * LW : (w + 1) * LW]
                    )
                    ot = otp.tile([P, LW], bf16, tag="ot")
                    for i in range(LC):
                        cs = slice(i * 512, (i + 1) * 512)
                        oh = ohp.tile([P, 512], bf16, tag="oh")
                        nc.vector.tensor_scalar(
                            out=oh[:],
                            in0=sh[:, cs],
                            scalar1=iotac[:, 0:1],
                            scalar2=None,
                            op0=OP.is_equal,
                        )
                        e_ps = ps_e.tile([P, 512], f32, tag="e")
                        nc.tensor.matmul(
                            e_ps[:],
                            lhsT=un_all[:, w * P : (w + 1) * P],
                            rhs=oh[:],
                            start=True,
                            stop=False,
                        )
                        nc.tensor.matmul(
                            e_ps[:], lhsT=identb[:], rhs=at[:, cs], start=False, stop=True
                        )
                        nc.scalar.activation(
                            out=ot[:, cs],
                            in_=e_ps[:],
                            func=AF.Identity,
                            bias=biasc_sb[:, 0:1],
                        )
                    nc.sync.dma_start(
                        out=outS_d.ap()[:, w * LW : (w + 1) * LW], in_=ot[:]
                    )

    nc.compile()
    return nc


def _prep_core_inputs(atom_embed, s, cond_to_s_idx, ln_gamma, ln_beta, W):
    """Host-side sharding + layout marshalling: transposes, LN param folding,
    and the sorted-window atom layout."""
    global _last_aux
    wg_full = (W * ln_gamma[None, :]).T.astype(np.float32)  # [C_S, C_ATOM]
    wg_host = np.ascontiguousarray(
        wg_full.reshape(KC, P, C_ATOM).transpose(1, 0, 2).reshape(P, KC * C_ATOM)
    ).astype(BF16)
    negw_host = np.ascontiguousarray(-wg_full.sum(axis=0).reshape(1, C_ATOM)).astype(
        BF16
    )
    biasc_host = np.ascontiguousarray(
        (W.astype(np.float32) @ ln_beta.astype(np.float32)).reshape(P, 1)
    )
    identb_host = np.eye(P, dtype=np.float32).astype(BF16)
    iotac_host = np.arange(P, dtype=np.float32).reshape(P, 1)

    in_maps = []
    aux = []
    for b in range(B):
        sT = np.ascontiguousarray(s[b].T.astype(np.float32))  # [C_S, N_RES]
        sT_host = np.ascontiguousarray(
            sT.reshape(KC, P, N_RES).transpose(1, 0, 2).reshape(P, KC * N_RES)
        ).astype(BF16)

        idxb = np.asarray(cond_to_s_idx[b]).astype(np.int64)  # values < 2048
        order = np.argsort(idxb)
        sidx = idxb[order]
        win = (sidx // WR).astype(np.int64)
        counts = np.bincount(win, minlength=NW)
        starts = np.zeros(NW, np.int64)
        starts[1:] = np.cumsum(counts)[:-1]

        cols = np.full(NCOLS, -1, dtype=np.int64)  # col -> atom id
        shifted = np.zeros(NW * LW, dtype=np.float16)
        clean_atoms, clean_idx = [], []
        for w in range(NW):
            n, st = int(counts[w]), int(starts[w])
            take = min(n, LW)
            cols[w * LW : w * LW + take] = order[st : st + take]
            shifted[w * LW : w * LW + take] = (sidx[st : st + take] - w * WR).astype(
                np.float16
            )
            if n > take:
                clean_atoms.extend(order[st + take : st + n].tolist())
                clean_idx.extend(sidx[st + take : st + n].tolist())
        assert len(clean_atoms) <= CL, (
            f"window overflow {len(clean_atoms)} > {CL}: indices too concentrated"
        )
        npad = CL - len(clean_atoms)
        cols[NW * LW :] = np.array(clean_atoms + [-1] * npad, dtype=np.int64)
        cl_idx = np.array(clean_idx + [0] * npad, dtype=np.int16)
        idxcl_host = np.ascontiguousarray(
            np.tile(np.ascontiguousarray(cl_idx.reshape(CL // 16, 16).T), (P // 16, 1))
        )

        atomT = atom_embed[b].T  # [C_ATOM, N_ATOMS] view
        atomS = np.zeros((P, NCOLS), dtype=np.float32)
        valid = cols >= 0
        atomS[:, valid] = atomT[:, cols[valid]]
        atomS_host = atomS.astype(BF16)

        shift_host = np.ascontiguousarray(
            np.broadcast_to(shifted[None, :], (P, NW * LW))
        )

        in_maps.append(
            {
                "sT": sT_host,
                "atomS": atomS_host,
                "shift": shift_host,
                "idxcl": idxcl_host,
                "wg": wg_host,
                "negw": negw_host,
                "biasc": biasc_host,
                "identb": identb_host,
                "iotac": iotac_host,
            }
        )
        aux.append((cols, valid))
    _last_aux = aux
    return in_maps


def _gather_output(res):
    out = np.empty((B, N_ATOMS, C_ATOM), dtype=np.float32)
    for b in range(B):
        cols, valid = _last_aux[b]
        outS = res.results[b]["outS"].astype(np.float32)  # [P, NCOLS]
        out[b][cols[valid], :] = outS[:, valid].T
    return out


def kernel(atom_embed, s, cond_to_s_idx, ln_gamma, ln_beta, W):
    global _compiled
    from concourse.bass_utils import run_bass_kernel_spmd

    atom_embed = np.asarray(atom_embed, dtype=np.float32)
    s = np.asarray(s, dtype=np.float32)
    cond_to_s_idx = np.asarray(cond_to_s_idx)
    ln_gamma = np.asarray(ln_gamma, dtype=np.float32)
    ln_beta = np.asarray(ln_beta, dtype=np.float32)
    W = np.asarray(W, dtype=np.float32)

    if _compiled is None:
        _compiled = _build()
    in_maps = _prep_core_inputs(atom_embed, s, cond_to_s_idx, ln_gamma, ln_beta, W)
    res = run_bass_kernel_spmd(_compiled, in_maps, core_ids=list(range(B)))
    return _gather_output(res)


# revision 15
# speedup vs baseline: 1014.6521x; 1.0763x over previous
"""Bass/Trainium2 kernel for nn_GatherUpdate: LayerNorm + Linear + per-atom
row gather + residual add, data-parallel over batch across 8 NeuronCores.

reference:
    normed = LayerNorm(s) * gamma + beta            # s: [B, 2048, 384]
    upd    = normed @ W.T                           # W: [128, 384] -> [B, 2048, 128]
    out    = atom_embed + upd[:, cond_to_s_idx, :]  # atom_embed: [B, 32768, 128]

Per-core plan (core b handles batch b). Everything runs in TRANSPOSED space
(c_atom on partitions). The LN+Linear is folded algebraically into matmuls:
upd^T = rstd*(Wg^T s^T) - rstd*mu*wgsum + bias (Wg/wgsum/bias host-folded).

The gather is restructured as a sorted one-hot EXPANSION on the PE. The host
sorts atoms by residue index and lays them out in 16 windows of 2560 output
columns, window w holding atoms whose residue is in [128w, 128w+128). For a
512-column chunk the gather is then one k=128 matmul:
    g[ca, j] = sum_r upd_nat[128w+r, ca] * onehot[r, j]
with onehot[r, j] = (shifted_idx[j] == r) built by one DVE is_equal per chunk
from a host-replicated fp16 shifted-index stream. The atom_embed residual is
accumulated into the same PSUM by an identity matmul, and the beta@W.T bias
rides the ACT PSUM->SBUF drain as a per-partition bias. Window overflow
(>2560 atoms on one 128-residue window; never happens for uniform indices)
falls back to a 512-slot GPSIMD ap_gather cleanup chunk.

This replaces the v1 on-chip ap_gather of all 32768 atoms (27.3 ns/idx ucode
= ~900 us/core measured) with ~35 us of PE work; measured HW total for v1 was
966 us vs 39 ms for the SWDGE row-gather baseline.
"""

import sys

sys.path.insert(0, "/opt/trn_rl_repo")

import numpy as np
import ml_dtypes

B = 8
N_ATOMS = 32768
N_RES = 2048
C_S = 384
C_ATOM = 128
EPS = 1e-5
P = 128
KC = C_S // P  # 3 contraction chunks
QN = 4  # 512-column chunks for stats/matmul/fixup
QS = N_RES // QN  # 512
NW = 16  # residue windows
WR = N_RES // NW  # 128 residues per window
LW = 2304  # output columns per window (2048 mean + 5.8 sigma slack)
CHUNKS = [512, 512, 512, 512, 256]  # per-window column chunks (sum=LW)
CL = 1024  # cleanup (overflow) columns, handled by ap_gather
NCOLS = NW * LW + CL  # 37888
BF16 = ml_dtypes.bfloat16

_compiled = None
_last_aux = None


def _build(repeat=1):
    """Build the per-core program. repeat>1 unrolls the whole pipeline N
    times (used only for timing: wall(N)-wall(1) cancels dispatch/transfer
    overhead)."""
    import concourse.bacc as bacc
    import concourse.tile as tile
    from concourse import mybir
    from concourse.masks import make_identity

    f32 = mybir.dt.float32
    bf16 = mybir.dt.bfloat16
    fp16 = mybir.dt.float16
    i16 = mybir.dt.int16
    AF = mybir.ActivationFunctionType
    OP = mybir.AluOpType

    nc = bacc.Bacc("TRN2", target_bir_lowering=False, debug=False)

    sT_d = nc.dram_tensor("sT", [P, KC * N_RES], bf16, kind="ExternalInput")
    atomS_d = nc.dram_tensor("atomS", [P, NCOLS], bf16, kind="ExternalInput")
    shift_d = nc.dram_tensor("shift", [P, NW * LW], fp16, kind="ExternalInput")
    idxcl_d = nc.dram_tensor("idxcl", [P, CL // 16], i16, kind="ExternalInput")
    wg_d = nc.dram_tensor("wg", [P, KC * C_ATOM], bf16, kind="ExternalInput")
    negw_d = nc.dram_tensor("negw", [1, C_ATOM], bf16, kind="ExternalInput")
    biasc_d = nc.dram_tensor("biasc", [P, 1], f32, kind="ExternalInput")
    iotac_d = nc.dram_tensor("iotac", [P, 1], f32, kind="ExternalInput")
    outS_d = nc.dram_tensor("outS", [P, NCOLS], bf16, kind="ExternalOutput")

    with tile.TileContext(nc) as tc:
        with (
            tc.tile_pool(name="consts", bufs=1) as consts,
            tc.tile_pool(name="sbig", bufs=1) as sbig,
            tc.tile_pool(name="rows", bufs=1) as rows,
            tc.tile_pool(name="t1p", bufs=2) as t1p,
            tc.tile_pool(name="updp", bufs=1) as updp,
            tc.tile_pool(name="ps_stat", bufs=1, space="PSUM") as ps_stat,
            tc.tile_pool(name="ps_a", bufs=2, space="PSUM") as ps_a,
            tc.tile_pool(name="ps_b", bufs=1, space="PSUM") as ps_b,
            tc.tile_pool(name="ps_t", bufs=1, space="PSUM") as ps_t,
            tc.tile_pool(name="ps_e", bufs=2, space="PSUM") as ps_e,
            tc.tile_pool(name="ohp", bufs=4) as ohp,
            tc.tile_pool(name="shp", bufs=3) as shp,
            tc.tile_pool(name="atp", bufs=3) as atp,
            tc.tile_pool(name="otp", bufs=3) as otp,
            tc.tile_pool(name="clp", bufs=1) as clp,
        ):
            # --- constants (loaded once) ---
            wg_sb = consts.tile([P, KC * C_ATOM], bf16)
            nc.sync.dma_start(out=wg_sb[:], in_=wg_d.ap())
            negw_sb = consts.tile([1, C_ATOM], bf16)
            nc.sync.dma_start(out=negw_sb[:], in_=negw_d.ap())
            biasc_sb = consts.tile([P, 1], f32)
            nc.sync.dma_start(out=biasc_sb[:], in_=biasc_d.ap())
            iotac = consts.tile([P, 1], f32)
            nc.sync.dma_start(out=iotac[:], in_=iotac_d.ap())
            idxcl_sb = consts.tile([P, CL // 16], i16)
            nc.sync.dma_start(out=idxcl_sb[:], in_=idxcl_d.ap())
            ones1 = consts.tile([1, P], bf16)
            nc.vector.memset(ones1[:], 1.0)
            onesc = consts.tile([P, 1], bf16)
            nc.vector.memset(onesc[:], 1.0)
            eps_t = consts.tile([1, 1], f32)
            nc.vector.memset(eps_t[:], EPS)
            ident32 = consts.tile([P, P], f32)
            make_identity(nc, ident32[:])

            for _rep in range(repeat):
                # --- loads: s first (it gates the critical path) ---
                sT = sbig.tile([P, KC * N_RES], bf16, tag="sT")
                for k in range(KC):
                    nc.sync.dma_start(
                        out=sT[:, k * N_RES : (k + 1) * N_RES],
                        in_=sT_d.ap()[:, k * N_RES : (k + 1) * N_RES],
                    )
                at_cl = clp.tile([P, CL], bf16, tag="atcl")
                nc.sync.dma_start(out=at_cl[:], in_=atomS_d.ap()[:, NW * LW :])

                # --- squares for E[x^2] ---
                sq = sbig.tile([P, KC * N_RES], bf16, tag="sq")
                for k in range(KC):
                    nc.scalar.activation(
                        out=sq[:, k * N_RES : (k + 1) * N_RES],
                        in_=sT[:, k * N_RES : (k + 1) * N_RES],
                        func=AF.Square,
                    )

                # --- LN folded into matmuls: updT = rstd*(Wg^T sT - wgsum x mu)
                mu_sb = rows.tile([1, N_RES], bf16, tag="mu")
                ex2e = rows.tile([1, N_RES], f32, tag="ex2e")
                musq = rows.tile([1, N_RES], f32, tag="musq")
                vare = rows.tile([1, N_RES], f32, tag="vare")
                rstd = rows.tile([1, N_RES], bf16, tag="rstd")
                updT = updp.tile([P, N_RES], f32, tag="updT")

                for q in range(QN):
                    qs = slice(q * QS, (q + 1) * QS)
                    mu_ps = ps_stat.tile([1, QS], f32, tag="mu")
                    sq_ps = ps_stat.tile([1, QS], f32, tag="sq")
                    for k in range(KC):
                        ks = slice(k * N_RES + q * QS, k * N_RES + (q + 1) * QS)
                        nc.tensor.matmul(
                            mu_ps[:],
                            lhsT=onesc[:],
                            rhs=sT[:, ks],
                            start=(k == 0),
                            stop=(k == KC - 1),
                        )
                    for k in range(KC):
                        ks = slice(k * N_RES + q * QS, k * N_RES + (q + 1) * QS)
                        nc.tensor.matmul(
                            sq_ps[:],
                            lhsT=onesc[:],
                            rhs=sq[:, ks],
                            start=(k == 0),
                            stop=(k == KC - 1),
                        )
                    nc.vector.tensor_scalar(
                        out=mu_sb[:, qs],
                        in0=mu_ps[:],
                        scalar1=1.0 / C_S,
                        scalar2=None,
                        op0=OP.mult,
                    )
                    nc.scalar.activation(
                        out=musq[:, qs], in_=mu_ps[:], func=AF.Square, scale=1.0 / C_S
                    )
                    nc.scalar.activation(
                        out=ex2e[:, qs],
                        in_=sq_ps[:],
                        func=AF.Identity,
                        bias=eps_t[:],
                        scale=1.0 / C_S,
                    )
                    nc.vector.scalar_tensor_tensor(
                        out=vare[:, qs],
                        in0=musq[:, qs],
                        scalar=-1.0,
                        in1=ex2e[:, qs],
                        op0=OP.mult,
                        op1=OP.add,
                    )
                    nc.scalar.activation(
                        out=rstd[:, qs], in_=vare[:, qs], func=AF.Abs_reciprocal_sqrt
                    )

                    a_ps = ps_a.tile([P, QS], f32, tag="A")
                    for k in range(KC):
                        ks = slice(k * N_RES + q * QS, k * N_RES + (q + 1) * QS)
                        nc.tensor.matmul(
                            a_ps[:],
                            lhsT=wg_sb[:, k * C_ATOM : (k + 1) * C_ATOM],
                            rhs=sT[:, ks],
                            start=(k == 0),
                            stop=False,
                        )
                    nc.tensor.matmul(
                        a_ps[:],
                        lhsT=negw_sb[:],
                        rhs=mu_sb[:, qs],
                        start=False,
                        stop=True,
                    )
                    b_ps = ps_b.tile([P, QS], f32, tag="t1b")
                    nc.tensor.matmul(
                        b_ps[:], lhsT=ones1[:], rhs=rstd[:, qs], start=True, stop=True
                    )
                    t1b_sb = t1p.tile([P, QS], f32, tag="t1bsb")
                    nc.scalar.copy(out=t1b_sb[:], in_=b_ps[:])
                    nc.vector.tensor_tensor(
                        out=updT[:, qs], in0=a_ps[:], in1=t1b_sb[:], op=OP.mult
                    )

                # --- overflow cleanup: ap_gather of <=512 leftover atoms ---
                g_cl = clp.tile([P, CL], f32, tag="gcl")
                nc.gpsimd.ap_gather(
                    g_cl[:], updT[:], idxcl_sb[:], P, N_RES, 1, CL
                )
                o_cl = clp.tile([P, CL], bf16, tag="ocl")
                nc.vector.scalar_tensor_tensor(
                    out=o_cl[:],
                    in0=at_cl[:],
                    scalar=biasc_sb[:, 0:1],
                    in1=g_cl[:],
                    op0=OP.add,
                    op1=OP.add,
                )
                nc.sync.dma_start(out=outS_d.ap()[:, NW * LW :], in_=o_cl[:])

                # --- transpose updT into natural-layout bf16 window weights ---
                un_all = updp.tile([P, NW * P], bf16, tag="un")
                for w in range(NW):
                    tr_ps = ps_t.tile([P, P], f32, tag="tr")
                    nc.tensor.transpose(
                        out=tr_ps[:],
                        in_=updT[:, w * WR : (w + 1) * WR],
                        identity=ident32[:],
                    )
                    nc.scalar.copy(out=un_all[:, w * P : (w + 1) * P], in_=tr_ps[:])

                # --- sorted one-hot expansion + residual add, per window ---
                for w in range(NW):
                    sh = shp.tile([P, LW], fp16, tag="sh")
                    nc.sync.dma_start(
                        out=sh[:], in_=shift_d.ap()[:, w * LW : (w + 1) * LW]
                    )
                    at = atp.tile([P, LW], bf16, tag="at")
                    nc.sync.dma_start(
                        out=at[:], in_=atomS_d.ap()[:, w * LW : (w + 1) * LW]
                    )
                    ot = otp.tile([P, LW], bf16, tag="ot")
                    off = 0
                    for ln in CHUNKS:
                        cs = slice(off, off + ln)
                        off += ln
                        oh = ohp.tile([P, 512], bf16, tag="oh")
                        nc.vector.tensor_scalar(
                            out=oh[:, :ln],
                            in0=sh[:, cs],
                            scalar1=iotac[:, 0:1],
                            scalar2=None,
                            op0=OP.is_equal,
                        )
                        e_ps = ps_e.tile([P, 512], f32, tag="e")
                        nc.tensor.matmul(
                            e_ps[:, :ln],
                            lhsT=un_all[:, w * P : (w + 1) * P],
                            rhs=oh[:, :ln],
                            start=True,
                            stop=True,
                        )
                        # fused PSUM drain + residual add + bias (one DVE pass)
                        nc.vector.scalar_tensor_tensor(
                            out=ot[:, cs],
                            in0=at[:, cs],
                            scalar=biasc_sb[:, 0:1],
                            in1=e_ps[:, :ln],
                            op0=OP.add,
                            op1=OP.add,
                        )
                    nc.sync.dma_start(
                        out=outS_d.ap()[:, w * LW : (w + 1) * LW], in_=ot[:]
                    )

    nc.compile()
    return nc


def _prep_core_inputs(atom_embed, s, cond_to_s_idx, ln_gamma, ln_beta, W):
    """Host-side sharding + layout marshalling: transposes, LN param folding,
    and the sorted-window atom layout."""
    global _last_aux
    wg_full = (W * ln_gamma[None, :]).T.astype(np.float32)  # [C_S, C_ATOM]
    wg_host = np.ascontiguousarray(
        wg_full.reshape(KC, P, C_ATOM).transpose(1, 0, 2).reshape(P, KC * C_ATOM)
    ).astype(BF16)
    negw_host = np.ascontiguousarray(-wg_full.sum(axis=0).reshape(1, C_ATOM)).astype(
        BF16
    )
    biasc_host = np.ascontiguousarray(
        (W.astype(np.float32) @ ln_beta.astype(np.float32)).reshape(P, 1)
    )
    iotac_host = np.arange(P, dtype=np.float32).reshape(P, 1)

    in_maps = []
    aux = []
    for b in range(B):
        sT = np.ascontiguousarray(s[b].T.astype(np.float32))  # [C_S, N_RES]
        sT_host = np.ascontiguousarray(
            sT.reshape(KC, P, N_RES).transpose(1, 0, 2).reshape(P, KC * N_RES)
        ).astype(BF16)

        idxb = np.asarray(cond_to_s_idx[b]).astype(np.int64)  # values < 2048
        order = np.argsort(idxb)
        sidx = idxb[order]
        win = (sidx // WR).astype(np.int64)
        counts = np.bincount(win, minlength=NW)
        starts = np.zeros(NW, np.int64)
        starts[1:] = np.cumsum(counts)[:-1]

        cols = np.full(NCOLS, -1, dtype=np.int64)  # col -> atom id
        shifted = np.zeros(NW * LW, dtype=np.float16)
        clean_atoms, clean_idx = [], []
        for w in range(NW):
            n, st = int(counts[w]), int(starts[w])
            take = min(n, LW)
            cols[w * LW : w * LW + take] = order[st : st + take]
            shifted[w * LW : w * LW + take] = (sidx[st : st + take] - w * WR).astype(
                np.float16
            )
            if n > take:
                clean_atoms.extend(order[st + take : st + n].tolist())
                clean_idx.extend(sidx[st + take : st + n].tolist())
        assert len(clean_atoms) <= CL, (
            f"window overflow {len(clean_atoms)} > {CL}: indices too concentrated"
        )
        npad = CL - len(clean_atoms)
        cols[NW * LW :] = np.array(clean_atoms + [-1] * npad, dtype=np.int64)
        cl_idx = np.array(clean_idx + [0] * npad, dtype=np.int16)
        idxcl_host = np.ascontiguousarray(
            np.tile(np.ascontiguousarray(cl_idx.reshape(CL // 16, 16).T), (P // 16, 1))
        )

        atomT = atom_embed[b].T  # [C_ATOM, N_ATOMS] view
        atomS = np.zeros((P, NCOLS), dtype=np.float32)
        valid = cols >= 0
        atomS[:, valid] = atomT[:, cols[valid]]
        atomS_host = atomS.astype(BF16)

        shift_host = np.ascontiguousarray(
            np.broadcast_to(shifted[None, :], (P, NW * LW))
        )

        in_maps.append(
            {
                "sT": sT_host,
                "atomS": atomS_host,
                "shift": shift_host,
                "idxcl": idxcl_host,
                "wg": wg_host,
                "negw": negw_host,
                "biasc": biasc_host,
                "iotac": iotac_host,
            }
        )
        aux.append((cols, valid))
    _last_aux = aux
    return in_maps


def _gather_output(res):
    out = np.empty((B, N_ATOMS, C_ATOM), dtype=np.float32)
    for b in range(B):
        cols, valid = _last_aux[b]
        outS = res.results[b]["outS"].astype(np.float32)  # [P, NCOLS]
        out[b][cols[valid], :] = outS[:, valid].T
    return out


def kernel(atom_embed, s, cond_to_s_idx, ln_gamma, ln_beta, W):
    global _compiled
    from concourse.bass_utils import run_bass_kernel_spmd

    atom_embed = np.asarray(atom_embed, dtype=np.float32)
    s = np.asarray(s, dtype=np.float32)
    cond_to_s_idx = np.asarray(cond_to_s_idx)
    ln_gamma = np.asarray(ln_gamma, dtype=np.float32)
    ln_beta = np.asarray(ln_beta, dtype=np.float32)
    W = np.asarray(W, dtype=np.float32)

    if _compiled is None:
        _compiled = _build()
    in_maps = _prep_core_inputs(atom_embed, s, cond_to_s_idx, ln_gamma, ln_beta, W)
    res = run_bass_kernel_spmd(_compiled, in_maps, core_ids=list(range(B)))
    return _gather_output(res)
